# revision 18
# baseline (speedup 1.0000x reference)
"""Batched Householder reflection: s_new[b] = s[b] - 2*(v[b]@s[b])/(v[b]@v[b]) * v[b].

Full inputs v, s: [512, 512] f32. Sharded batch-parallel across 8 NeuronCores
(64 rows per core). All I/O in bf16 (rel-err gate is 2e-2; bf16 end-to-end
lands ~2.4e-3): halves DMA bytes. Compute speed is dtype-independent here
(STT/activation have no DVE 2x perf mode), so bf16 only buys DMA time.

Per core one [64, 2, 512] bf16 tile: row b holds v[b] | s[b] on partition b.
Dynamic-DMA queues are the bottleneck (~40-60 GB/s per queue), so the 128KB
load is fanned across all 4 queues (SP + ACT HWDGE, 2 Pool SWDGE), skewed so
the late-starting Pool queues carry less. Store is split SP/ACT.

  dot = rowsum(v*s)   DVE scalar_tensor_tensor accum_out
  nsq = rowsum(v*v)   ACT Square activation accum_out (parallel with dot)
  rcp = 1/nsq; coef = -2*dot*rcp; out = coef*v + s (DVE)
"""

import numpy as np

B, K = 512, 512
N_CORES = 8
B_LOC = B // N_CORES  # 64 rows per core

# load split row boundaries: SP / ACT (HWDGE only; SWDGE engines 7/15
# straggle by ~2.7us under descriptor-ring port contention). SP issues
# ~230ns before ACT, so it carries a few more rows.
LS = (0, 36, 64)

_nc = None


def _build():
    import concourse.bass as bass
    from concourse import mybir

    nc = bass.Bass("TRN2", debug=False, num_devices=N_CORES, num_swdge_queues=1)
    bf16 = mybir.dt.bfloat16
    f32 = mybir.dt.float32

    # Preamble surgery: drop the framework's const-tile MEMSETs (unused
    # here), the post-init all-engine barrier (the runtime's own engine
    # barrier right before `main` already orders everything this kernel
    # needs), and every Pool/PE instruction -- the kernel never uses those
    # engines, and an engine with no instructions drops out of the NEFF's
    # serialized init-barrier rounds.
    bb = nc.main_func.blocks[0]
    keep = [
        i
        for i in bb.instructions
        if type(i).__name__ not in ("InstMemset", "InstDrain", "InstEventSemaphore")
        and getattr(i, "engine", None)
        not in (mybir.EngineType.Pool, mybir.EngineType.PE)
    ]
    del bb.instructions[:]
    bb.instructions.extend(keep)

    x = nc.dram_tensor("x", [B_LOC, 2, K], bf16, kind="ExternalInput").ap()
    xt = nc.alloc_sbuf_tensor("xt", [B_LOC, 2, K], bf16).ap()
    v_t = xt[:, 0, :]
    s_t = xt[:, 1, :]

    out = nc.dram_tensor("out", [B_LOC, K], bf16, kind="ExternalOutput").ap()

    o_t = nc.alloc_sbuf_tensor("o_t", [B_LOC, K], bf16).ap()
    junk_vs = nc.alloc_sbuf_tensor("junk_vs", [B_LOC, K], bf16).ap()
    junk_vv = nc.alloc_sbuf_tensor("junk_vv", [B_LOC, K], bf16).ap()
    warm = nc.alloc_sbuf_tensor("warm", [B_LOC, 1], f32).ap()
    dot = nc.alloc_sbuf_tensor("dot", [B_LOC, 1], f32).ap()
    nsq = nc.alloc_sbuf_tensor("nsq", [B_LOC, 1], f32).ap()
    rcp = nc.alloc_sbuf_tensor("rcp", [B_LOC, 1], f32).ap()
    coef = nc.alloc_sbuf_tensor("coef", [B_LOC, 1], f32).ap()

    dma_in = nc.alloc_semaphore("dma_in")
    act_done = nc.alloc_semaphore("act_done")
    dve_done = nc.alloc_semaphore("dve_done")
    act_issued = nc.alloc_semaphore("act_issued")
    dma_out = nc.alloc_semaphore("dma_out")

    mult = mybir.AluOpType.mult
    add = mybir.AluOpType.add
    Square = mybir.ActivationFunctionType.Square

    sp, act, ve = nc.sync, nc.scalar, nc.vector

    # ---- load: 2 HWDGE queues (SP, ACT) ----
    a, b, c = LS
    sp.dma_start(out=xt[a:b], in_=x[a:b]).then_inc(dma_in, 16)
    # dma_out carries the previous execution's (unwaited) store completions;
    # the runtime drained those rings before relaunching, so clear it here
    # (after the load issue -- the two HWDGE generators serialize, and SP's
    # bigger chunk should go first).
    sp.sem_clear(dma_out)
    act.dma_start(out=xt[b:c], in_=x[b:c]).then_inc(dma_in, 16)

    # ACT: prewarm the Square table while the DMAs fly (input is garbage)
    act.activation(out=warm[:], in_=dot[:], func=Square)
    act.wait_ge(dma_in, 32)
    act.activation(out=junk_vv[:], in_=v_t, func=Square, accum_out=nsq[:]).then_inc(
        act_done, 1
    )

    # DVE chain
    ve.wait_ge(dma_in, 32)
    ve.scalar_tensor_tensor(
        out=junk_vs[:],
        in0=v_t,
        scalar=1.0,
        in1=s_t,
        op0=mult,
        op1=mult,
        accum_out=dot[:],
    )
    # DVE writes are not visible to the next DVE instruction without a
    # completion barrier (in-order issue != in-order write visibility).
    # drain() blocks the sequencer until the engine pipe empties (writes
    # committed) -- cheaper than a semaphore round-trip for same-engine deps.
    ve.drain()
    ve.wait_ge(act_done, 1)
    ve.reciprocal(out=rcp[:], in_=nsq[:])
    ve.drain()
    ve.scalar_tensor_tensor(
        out=coef[:], in0=dot[:], scalar=-2.0, in1=rcp[:], op0=mult, op1=mult
    )
    ve.drain()
    # Final op split asymmetrically by K: a small first chunk lets SP's
    # store issue start early and finish inside the second chunk's window.
    KA = 160
    ve.scalar_tensor_tensor(
        out=o_t[:, :KA],
        in0=v_t[:, :KA],
        scalar=coef[:],
        in1=s_t[:, :KA],
        op0=mult,
        op1=add,
    ).then_inc(dve_done, 1)
    ve.scalar_tensor_tensor(
        out=o_t[:, KA:],
        in0=v_t[:, KA:],
        scalar=coef[:],
        in1=s_t[:, KA:],
        op0=mult,
        op1=add,
    ).then_inc(dve_done, 1)

    # ---- store: K-chunks on SP (early, hidden under the second final op)
    # and ACT (late); the sequencers never wait for the store to land (the
    # runtime drains DMA rings at execution end).
    sp.wait_ge(dve_done, 1)
    sp.dma_start(out=out[:, :KA], in_=o_t[:, :KA]).then_inc(dma_out, 16)
    act.wait_ge(dve_done, 2)
    act.sem_inc(act_issued, 1)
    act.dma_start(out=out[:, KA:], in_=o_t[:, KA:]).then_inc(dma_out, 16)

    # SP resets semaphores for re-execution (PJRT reuses the loaded NEFF;
    # semaphores persist between executions). SP's dve_done wait proves DVE
    # (and hence ACT's square) passed the early sems; act_issued proves ACT
    # passed its dve_done wait, so dve_done is safe to clear.
    for sem in (dma_in, act_done):
        sp.sem_clear(sem)
    sp.wait_ge(act_issued, 1)
    sp.sem_clear(dve_done)
    sp.sem_clear(act_issued)

    return nc


def _shards(v, s):
    import ml_dtypes

    bf16 = ml_dtypes.bfloat16
    v = np.asarray(v, dtype=np.float32).astype(bf16)
    s = np.asarray(s, dtype=np.float32).astype(bf16)
    maps = []
    for c in range(N_CORES):
        vc = v[c * B_LOC : (c + 1) * B_LOC]
        sc = s[c * B_LOC : (c + 1) * B_LOC]
        maps.append({"x": np.ascontiguousarray(np.stack([vc, sc], axis=1))})
    return maps


def kernel(i=None, v=None, s=None, **_):
    global _nc
    from concourse.bass_utils import run_bass_kernel_spmd

    if _nc is None:
        _nc = _build()

    in_maps = _shards(v, s)
    res = run_bass_kernel_spmd(_nc, in_maps, core_ids=list(range(N_CORES)))
    return np.concatenate([r["out"] for r in res.results], axis=0).astype(np.float32)


# revision 19
# speedup vs baseline: 1.1932x; 1.1932x over previous
"""Batched Householder reflection: s_new[b] = s[b] - 2*(v[b]@s[b])/(v[b]@v[b]) * v[b].

Full inputs v, s: [512, 512] f32. Sharded batch-parallel across 8 NeuronCores
(64 rows per core). All I/O in bf16 (rel-err gate is 2e-2; bf16 end-to-end
lands ~2.4e-3): halves DMA bytes. Compute speed is dtype-independent here
(STT/activation have no DVE 2x perf mode), so bf16 only buys DMA time.

Per core one [64, 2, 512] bf16 tile: row b holds v[b] | s[b] on partition b.
The 128KB load is split across the two HWDGE queues (SP + ACT; Pool/SWDGE
queues straggle by microseconds under descriptor-ring contention). The
framework preamble is trimmed (const MEMSETs, post-init barrier, Pool/PE
register inits deleted), stores are issued without a completion wait (the
runtime drains DMA rings at execution end), and the final op is split so the
first store issue hides under the second chunk's compute.

  dot = rowsum(v*s)   DVE scalar_tensor_tensor accum_out
  nsq = rowsum(v*v)   ACT Square activation accum_out (parallel with dot)
  rcp = 1/nsq; coef = -2*dot*rcp; out = coef*v + s (DVE)
"""

import numpy as np

B, K = 512, 512
N_CORES = 8
B_LOC = B // N_CORES  # 64 rows per core

# load split row boundaries: SP / ACT (HWDGE only; SWDGE engines 7/15
# straggle by ~2.7us under descriptor-ring port contention). SP issues
# ~230ns before ACT, so it carries a few more rows.
LS = (0, 36, 64)

_nc = None


def _build():
    import concourse.bass as bass
    from concourse import mybir

    nc = bass.Bass("TRN2", debug=False, num_devices=N_CORES, num_swdge_queues=1)
    bf16 = mybir.dt.bfloat16
    f32 = mybir.dt.float32

    # Preamble surgery: drop the framework's const-tile MEMSETs (unused
    # here), the post-init all-engine barrier (the runtime's own engine
    # barrier right before `main` already orders everything this kernel
    # needs), and every Pool/PE instruction -- the kernel never uses those
    # engines, and an engine with no instructions drops out of the NEFF's
    # serialized init-barrier rounds.
    bb = nc.main_func.blocks[0]
    keep = [
        i
        for i in bb.instructions
        if type(i).__name__ not in ("InstMemset", "InstDrain", "InstEventSemaphore")
        and getattr(i, "engine", None)
        not in (mybir.EngineType.Pool, mybir.EngineType.PE)
    ]
    del bb.instructions[:]
    bb.instructions.extend(keep)

    x = nc.dram_tensor("x", [B_LOC, 2, K], bf16, kind="ExternalInput").ap()
    xt = nc.alloc_sbuf_tensor("xt", [B_LOC, 2, K], bf16).ap()
    v_t = xt[:, 0, :]
    s_t = xt[:, 1, :]

    out = nc.dram_tensor("out", [B_LOC, K], bf16, kind="ExternalOutput").ap()

    o_t = nc.alloc_sbuf_tensor("o_t", [B_LOC, K], bf16).ap()
    junk_vs = nc.alloc_sbuf_tensor("junk_vs", [B_LOC, K], bf16).ap()
    junk_vv = nc.alloc_sbuf_tensor("junk_vv", [B_LOC, K], bf16).ap()
    warm = nc.alloc_sbuf_tensor("warm", [B_LOC, 1], f32).ap()
    dot = nc.alloc_sbuf_tensor("dot", [B_LOC, 1], f32).ap()
    nsq = nc.alloc_sbuf_tensor("nsq", [B_LOC, 1], f32).ap()
    rcp = nc.alloc_sbuf_tensor("rcp", [B_LOC, 1], f32).ap()
    coef = nc.alloc_sbuf_tensor("coef", [B_LOC, 1], f32).ap()

    dma_in = nc.alloc_semaphore("dma_in")
    act_done = nc.alloc_semaphore("act_done")
    dve_done = nc.alloc_semaphore("dve_done")
    act_issued = nc.alloc_semaphore("act_issued")
    dma_out = nc.alloc_semaphore("dma_out")

    mult = mybir.AluOpType.mult
    add = mybir.AluOpType.add
    Square = mybir.ActivationFunctionType.Square

    sp, act, ve = nc.sync, nc.scalar, nc.vector

    # ---- load: 2 HWDGE queues (SP, ACT) ----
    a, b, c = LS
    sp.dma_start(out=xt[a:b], in_=x[a:b]).then_inc(dma_in, 16)
    # dma_out carries the previous execution's (unwaited) store completions;
    # the runtime drained those rings before relaunching, so clear it here
    # (after the load issue -- the two HWDGE generators serialize, and SP's
    # bigger chunk should go first).
    sp.sem_clear(dma_out)
    act.dma_start(out=xt[b:c], in_=x[b:c]).then_inc(dma_in, 16)

    # ACT: prewarm the Square table while the DMAs fly (input is garbage)
    act.activation(out=warm[:], in_=dot[:], func=Square)
    act.wait_ge(dma_in, 32)
    act.activation(out=junk_vv[:], in_=v_t, func=Square, accum_out=nsq[:]).then_inc(
        act_done, 1
    )

    # DVE chain
    ve.wait_ge(dma_in, 32)
    ve.scalar_tensor_tensor(
        out=junk_vs[:],
        in0=v_t,
        scalar=1.0,
        in1=s_t,
        op0=mult,
        op1=mult,
        accum_out=dot[:],
    )
    # DVE writes are not visible to the next DVE instruction without a
    # completion barrier (in-order issue != in-order write visibility).
    # drain() blocks the sequencer until the engine pipe empties (writes
    # committed) -- cheaper than a semaphore round-trip for same-engine deps.
    ve.drain()
    ve.wait_ge(act_done, 1)
    ve.reciprocal(out=rcp[:], in_=nsq[:])
    ve.drain()
    ve.scalar_tensor_tensor(
        out=coef[:], in0=dot[:], scalar=-2.0, in1=rcp[:], op0=mult, op1=mult
    )
    ve.drain()
    # Final op split asymmetrically by K: a small first chunk lets SP's
    # store issue start early and finish inside the second chunk's window.
    KA = 160
    ve.scalar_tensor_tensor(
        out=o_t[:, :KA],
        in0=v_t[:, :KA],
        scalar=coef[:],
        in1=s_t[:, :KA],
        op0=mult,
        op1=add,
    ).then_inc(dve_done, 1)
    ve.scalar_tensor_tensor(
        out=o_t[:, KA:],
        in0=v_t[:, KA:],
        scalar=coef[:],
        in1=s_t[:, KA:],
        op0=mult,
        op1=add,
    ).then_inc(dve_done, 1)

    # ---- store: K-chunks on SP (early, hidden under the second final op)
    # and ACT (late); the sequencers never wait for the store to land (the
    # runtime drains DMA rings at execution end).
    sp.wait_ge(dve_done, 1)
    sp.dma_start(out=out[:, :KA], in_=o_t[:, :KA]).then_inc(dma_out, 16)
    act.wait_ge(dve_done, 2)
    act.sem_inc(act_issued, 1)
    act.dma_start(out=out[:, KA:], in_=o_t[:, KA:]).then_inc(dma_out, 16)

    # SP resets semaphores for re-execution (PJRT reuses the loaded NEFF;
    # semaphores persist between executions). SP's dve_done wait proves DVE
    # (and hence ACT's square) passed the early sems; act_issued proves ACT
    # passed its dve_done wait, so dve_done is safe to clear.
    for sem in (dma_in, act_done):
        sp.sem_clear(sem)
    sp.wait_ge(act_issued, 1)
    sp.sem_clear(dve_done)
    sp.sem_clear(act_issued)

    return nc


def _shards(v, s):
    import ml_dtypes

    bf16 = ml_dtypes.bfloat16
    v = np.asarray(v, dtype=np.float32).astype(bf16)
    s = np.asarray(s, dtype=np.float32).astype(bf16)
    maps = []
    for c in range(N_CORES):
        vc = v[c * B_LOC : (c + 1) * B_LOC]
        sc = s[c * B_LOC : (c + 1) * B_LOC]
        maps.append({"x": np.ascontiguousarray(np.stack([vc, sc], axis=1))})
    return maps


def kernel(i=None, v=None, s=None, **_):
    global _nc
    from concourse.bass_utils import run_bass_kernel_spmd

    if _nc is None:
        _nc = _build()

    in_maps = _shards(v, s)
    res = run_bass_kernel_spmd(_nc, in_maps, core_ids=list(range(N_CORES)))
    return np.concatenate([r["out"] for r in res.results], axis=0).astype(np.float32)


# revision 20
# speedup vs baseline: 1.1981x; 1.0041x over previous
"""Batched Householder reflection: s_new[b] = s[b] - 2*(v[b]@s[b])/(v[b]@v[b]) * v[b].

Full inputs v, s: [512, 512] f32. Sharded batch-parallel across 8 NeuronCores
(64 rows per core). All I/O in bf16 (rel-err gate is 2e-2; bf16 end-to-end
lands ~2.4e-3): halves DMA bytes. Compute speed is dtype-independent here
(STT/activation have no DVE 2x perf mode), so bf16 only buys DMA time.

Per core one [64, 2, 512] bf16 tile: row b holds v[b] | s[b] on partition b.
The 128KB load is split across the two HWDGE queues (SP + ACT; Pool/SWDGE
queues straggle by microseconds under descriptor-ring contention). The
framework preamble is trimmed (const MEMSETs, post-init barrier, Pool/PE
register inits deleted), stores are issued without a completion wait (the
runtime drains DMA rings at execution end), and the final op is split so the
first store issue hides under the second chunk's compute.

  dot = rowsum(v*s)   DVE scalar_tensor_tensor accum_out
  nsq = rowsum(v*v)   ACT Square activation accum_out (parallel with dot)
  rcp = 1/nsq; coef = -2*dot*rcp; out = coef*v + s (DVE)
"""

import numpy as np

B, K = 512, 512
N_CORES = 8
B_LOC = B // N_CORES  # 64 rows per core

# load split row boundaries: SP / ACT (HWDGE only; SWDGE engines 7/15
# straggle by ~2.7us under descriptor-ring port contention). SP issues
# ~230ns before ACT, so it carries a few more rows.
LS = (0, 36, 64)

_nc = None


def _build():
    import concourse.bass as bass
    from concourse import mybir

    nc = bass.Bass("TRN2", debug=False, num_devices=N_CORES, num_swdge_queues=1)
    bf16 = mybir.dt.bfloat16
    f32 = mybir.dt.float32

    # Preamble surgery: drop the framework's const-tile MEMSETs (unused
    # here), the post-init all-engine barrier (the runtime's own engine
    # barrier right before `main` already orders everything this kernel
    # needs), and every Pool/PE instruction -- the kernel never uses those
    # engines, and an engine with no instructions drops out of the NEFF's
    # serialized init-barrier rounds.
    bb = nc.main_func.blocks[0]
    keep = [
        i
        for i in bb.instructions
        if type(i).__name__
        not in ("InstMemset", "InstDrain", "InstEventSemaphore", "InstRegisterMove")
        and getattr(i, "engine", None)
        not in (mybir.EngineType.Pool, mybir.EngineType.PE)
    ]
    del bb.instructions[:]
    bb.instructions.extend(keep)

    x = nc.dram_tensor("x", [B_LOC, 2, K], bf16, kind="ExternalInput").ap()
    xt = nc.alloc_sbuf_tensor("xt", [B_LOC, 2, K], bf16).ap()
    v_t = xt[:, 0, :]
    s_t = xt[:, 1, :]

    out = nc.dram_tensor("out", [B_LOC, K], bf16, kind="ExternalOutput").ap()

    o_t = nc.alloc_sbuf_tensor("o_t", [B_LOC, K], bf16).ap()
    junk_vs = nc.alloc_sbuf_tensor("junk_vs", [B_LOC, K], bf16).ap()
    junk_vv = nc.alloc_sbuf_tensor("junk_vv", [B_LOC, K], bf16).ap()
    warm = nc.alloc_sbuf_tensor("warm", [B_LOC, 1], f32).ap()
    dot = nc.alloc_sbuf_tensor("dot", [B_LOC, 1], f32).ap()
    nsq = nc.alloc_sbuf_tensor("nsq", [B_LOC, 1], f32).ap()
    rcp = nc.alloc_sbuf_tensor("rcp", [B_LOC, 1], f32).ap()
    coef = nc.alloc_sbuf_tensor("coef", [B_LOC, 1], f32).ap()

    dma_in = nc.alloc_semaphore("dma_in")
    act_done = nc.alloc_semaphore("act_done")
    dve_done = nc.alloc_semaphore("dve_done")
    act_issued = nc.alloc_semaphore("act_issued")
    dma_out = nc.alloc_semaphore("dma_out")

    mult = mybir.AluOpType.mult
    add = mybir.AluOpType.add
    Square = mybir.ActivationFunctionType.Square

    sp, act, ve = nc.sync, nc.scalar, nc.vector

    # ---- load: 2 HWDGE queues (SP, ACT) ----
    a, b, c = LS
    sp.dma_start(out=xt[a:b], in_=x[a:b]).then_inc(dma_in, 16)
    # dma_out carries the previous execution's (unwaited) store completions;
    # the runtime drained those rings before relaunching, so clear it here
    # (after the load issue -- the two HWDGE generators serialize, and SP's
    # bigger chunk should go first).
    sp.sem_clear(dma_out)
    act.dma_start(out=xt[b:c], in_=x[b:c]).then_inc(dma_in, 16)

    # ACT: prewarm the Square table while the DMAs fly (input is garbage)
    act.activation(out=warm[:], in_=dot[:], func=Square)
    act.wait_ge(dma_in, 32)
    act.activation(out=junk_vv[:], in_=v_t, func=Square, accum_out=nsq[:]).then_inc(
        act_done, 1
    )

    # DVE chain
    ve.wait_ge(dma_in, 32)
    ve.scalar_tensor_tensor(
        out=junk_vs[:],
        in0=v_t,
        scalar=1.0,
        in1=s_t,
        op0=mult,
        op1=mult,
        accum_out=dot[:],
    )
    # DVE writes are not visible to the next DVE instruction without a
    # completion barrier (in-order issue != in-order write visibility).
    # drain() blocks the sequencer until the engine pipe empties (writes
    # committed) -- cheaper than a semaphore round-trip for same-engine
    # deps. The drain after rcp also covers the dot accumulator read (pipe
    # empty implies every prior write committed).
    ve.wait_ge(act_done, 1)
    ve.reciprocal(out=rcp[:], in_=nsq[:])
    ve.drain()
    ve.scalar_tensor_tensor(
        out=coef[:], in0=dot[:], scalar=-2.0, in1=rcp[:], op0=mult, op1=mult
    )
    ve.drain()
    # Final op split asymmetrically by K: a small first chunk lets SP's
    # store issue start early and finish inside the second chunk's window.
    KA = 160
    ve.scalar_tensor_tensor(
        out=o_t[:, :KA],
        in0=v_t[:, :KA],
        scalar=coef[:],
        in1=s_t[:, :KA],
        op0=mult,
        op1=add,
    ).then_inc(dve_done, 1)
    ve.scalar_tensor_tensor(
        out=o_t[:, KA:],
        in0=v_t[:, KA:],
        scalar=coef[:],
        in1=s_t[:, KA:],
        op0=mult,
        op1=add,
    ).then_inc(dve_done, 1)

    # ---- store: K-chunks on SP (early, hidden under the second final op)
    # and ACT (late); the sequencers never wait for the store to land (the
    # runtime drains DMA rings at execution end).
    sp.wait_ge(dve_done, 1)
    sp.dma_start(out=out[:, :KA], in_=o_t[:, :KA]).then_inc(dma_out, 16)
    act.wait_ge(dve_done, 2)
    act.sem_inc(act_issued, 1)
    act.dma_start(out=out[:, KA:], in_=o_t[:, KA:]).then_inc(dma_out, 16)

    # SP resets semaphores for re-execution (PJRT reuses the loaded NEFF;
    # semaphores persist between executions). SP's dve_done wait proves DVE
    # (and hence ACT's square) passed the early sems; act_issued proves ACT
    # passed its dve_done wait, so dve_done is safe to clear.
    for sem in (dma_in, act_done):
        sp.sem_clear(sem)
    sp.wait_ge(act_issued, 1)
    sp.sem_clear(dve_done)
    sp.sem_clear(act_issued)

    return nc


def _shards(v, s):
    import ml_dtypes

    bf16 = ml_dtypes.bfloat16
    v = np.asarray(v, dtype=np.float32).astype(bf16)
    s = np.asarray(s, dtype=np.float32).astype(bf16)
    maps = []
    for c in range(N_CORES):
        vc = v[c * B_LOC : (c + 1) * B_LOC]
        sc = s[c * B_LOC : (c + 1) * B_LOC]
        maps.append({"x": np.ascontiguousarray(np.stack([vc, sc], axis=1))})
    return maps


def kernel(i=None, v=None, s=None, **_):
    global _nc
    from concourse.bass_utils import run_bass_kernel_spmd

    if _nc is None:
        _nc = _build()

    in_maps = _shards(v, s)
    res = run_bass_kernel_spmd(_nc, in_maps, core_ids=list(range(N_CORES)))
    return np.concatenate([r["out"] for r in res.results], axis=0).astype(np.float32)
